# revision 1
# baseline (speedup 1.0000x reference)
"""Trainium2 Bass kernel for nn_CaduceusEmbeddingsSTFT.

out[b, t, :] = concat(emb_table[ids[b, t]],
                      proj(|STFT(onehot(ids[b]))| upsampled at frame f(t)))

Structure exploited:
  * nearest upsampling -> only 129 distinct STFT frame rows per batch; the
    (8192 x 2064) @ (2064 x 154) projection collapses to (129 x 2064) @
    (2064 x 154) plus a row broadcast.
  * STFT of one-hot signals: windowed frames are one-hot masks, so
    spec = onehot_frames @ (window * DFT) as matmuls (cos / sin).
  * embedding lookup and frame broadcast are one-hot matmuls on the PE.
  * the sin row for k=0 is identically zero, so the nyquist cos row rides
    in column 0 of the im matmul; zeroing t2 row 0 after |ny| is read
    makes the full-tile add + sqrt yield |dc| in row 0 for free.

Precision: harness gate is rel 2e-2; plain bf16 operands with fp32 PSUM
accumulation land ~5e-3, so no hi/lo splits.

Schedule (v3): the kernel is output-DMA-bound (8.4 MB f32 per core at
~360 GB/s fair share ~= 23 us). Everything else hides under that stream:
  * emb drains go to DVE (plus ACT for the late groups, after its mag
    work) so the out-DMA starts early and never starves;
  * out-DMAs are grouped so the first/last transfers are small (fast
    ramp, short tail) and mid-kernel ones large (few issues);
  * all DMA issues on Sync — ACT must not stall on a backpressured
    HWDGE ring while it still has drains to run;
  * mag chain: squares/sqrts on ACT, row-0 zero on DVE, adds on GpSimd.

Sharding: 8 cores = 4 batches x 2 sequence halves; each core computes a
(4096, 512) output shard; boundary frame recomputed by both halves.
"""

import numpy as np

V = 16
D_EMB = 358
D_STFT = 154
NFFT = 256
HOP = 64
NFREQ = 129
B, L = 4, 8192
LH = L // 2  # 4096 rows per core
F = 65  # frames per core (inclusive overlap frame)
VF = V * F  # 1040
DM = 512
NCORES = 8
NT = LH // 128  # 32 output tiles per core
# output tile groups (one SBUF buffer + one DMA each)
EGROUPS = [4] * 8
SGROUPS = [8, 8, 8, 8]
# (start, size) chunks over the VF axis; multiples of F so projection
# lhsT slices [:, v*F:(v+1)*F] never cross a chunk boundary.
CHUNKS = [(0, 7 * F), (7 * F, 7 * F), (14 * F, 2 * F)]

_PROG = None
LAST_RESULT = None  # BassKernelResults of the most recent run (for harnesses)


def _build_program():
    import concourse.mybir as mybir
    import concourse.tile as tile
    from concourse import bacc

    f32 = mybir.dt.float32
    bf16 = mybir.dt.bfloat16
    i8 = mybir.dt.int8
    AO = mybir.AluOpType

    nc = bacc.Bacc("TRN2", target_bir_lowering=False, debug=False,
                   num_devices=NCORES)

    CWW = 2 * 128  # 256: per-c block width in cw (cos 128 | ny+sin 128)
    # packed int8 ids block: [h_emb 1024 | vemb 1 | h_frames 130 | vfr 16]
    IPW = LH // 4 + 1 + 2 * F + V
    # packed bf16 consts: [embrep 358 | cw 2*256 | wnyq 3 blocks of 154]
    APW = D_EMB + 2 * CWW + 3 * D_STFT

    ipack = nc.dram_tensor("ipack", [128, IPW], i8, kind="ExternalInput")
    apack = nc.dram_tensor("apack", [128, APW], bf16, kind="ExternalInput")
    wproj = nc.dram_tensor("wproj", [128, V * D_STFT], bf16,
                           kind="ExternalInput")
    bsel = nc.dram_tensor("bsel", [128, LH], bf16, kind="ExternalInput")
    out = nc.dram_tensor("out", [LH, DM], f32, kind="ExternalOutput")

    with tile.TileContext(nc) as tc:
        with (
            tc.tile_pool(name="consts", bufs=1) as cpool,
            tc.tile_pool(name="work", bufs=1) as wpool,
            tc.tile_pool(name="tmp", bufs=2) as tpool,
            tc.tile_pool(name="oemb", bufs=4) as oepool,
            tc.tile_pool(name="ostft", bufs=4) as ospool,
        ):
            # ---- const loads (issue order = need order) ---------------------
            IP = cpool.tile([128, IPW], i8, tag="ip")
            nc.sync.dma_start(out=IP[:], in_=ipack[:])
            AP_ = cpool.tile([128, APW], bf16, tag="ap")
            nc.sync.dma_start(out=AP_[:], in_=apack[:])
            WP = cpool.tile([128, V * D_STFT], bf16, tag="wp")
            nc.sync.dma_start(out=WP[:], in_=wproj[:])
            BS = cpool.tile([128, LH], bf16, tag="bs")
            nc.sync.dma_start(out=BS[:], in_=bsel[:])

            HE = IP[:, :LH // 4]
            VEMB = IP[:, LH // 4:LH // 4 + 1]
            HF = IP[:, LH // 4 + 1:LH // 4 + 1 + 2 * F]
            VFR = IP[:, LH // 4 + 1 + 2 * F:]
            ER = AP_[:, :D_EMB]
            CW = AP_[:, D_EMB:D_EMB + 2 * CWW]
            WN = AP_[0:7, D_EMB + 2 * CWW:]

            # ---- one-hot builds (bf16 out: 0/1 exact), normal priority so
            # the DFT can start early -----------------------------------------
            OHE = wpool.tile([128, LH // 4], bf16, tag="ohe")
            nc.vector.tensor_tensor(
                out=OHE[:], in0=HE, in1=VEMB.to_broadcast([128, LH // 4]),
                op=AO.is_equal)
            OHF = []
            for c in range(2):
                t = wpool.tile([128, VF], bf16, tag=f"ohf{c}")
                in0 = (HF[:, c * F:(c + 1) * F]
                       .rearrange("p (one f) -> p one f", one=1)
                       .to_broadcast([128, V, F]))
                in1 = (VFR.rearrange("p (v one) -> p v one", one=1)
                       .to_broadcast([128, V, F]))
                nc.vector.tensor_tensor(
                    out=t[:].rearrange("p (v f) -> p v f", v=V),
                    in0=in0, in1=in1, op=AO.is_equal)
                OHF.append(t)
            # static bits of the nyquist fold + SH zero padding, off the
            # critical path
            NYH = wpool.tile([1, VF + F], bf16, tag="nyh")
            nc.vector.memset(NYH[:, VF:], 1.0)
            SH = wpool.tile([128, D_STFT], bf16, tag="sh")
            nc.vector.memset(SH[:], 0.0)

            MAGH = wpool.tile([128, VF], bf16, tag="magh")

            with (
                tc.tile_pool(name="psum_re", bufs=2, space="PSUM") as pre,
                tc.tile_pool(name="psum_im", bufs=1, space="PSUM") as pim,
                tc.tile_pool(name="psum_emb", bufs=4, space="PSUM") as pemb,
                tc.tile_pool(name="psum_s", bufs=1, space="PSUM") as psp,
            ):
                # ---- emb pipeline: starts as soon as HE/VEMB/ER land --------
                ti0 = 0
                for gi, gn in enumerate(EGROUPS):
                    oe = oepool.tile([128, gn * D_EMB], f32, tag=f"oe{gn}")
                    for a in range(gn):
                        ti = ti0 + a
                        q, sub = divmod(ti, 4)
                        po = pemb.tile([128, D_EMB], f32, tag="pe")
                        nc.tensor.matmul(
                            out=po[:],
                            lhsT=OHE[32 * sub:32 * (sub + 1),
                                     q * 128:(q + 1) * 128],
                            rhs=ER[32 * sub:32 * (sub + 1), :],
                            start=True, stop=True,
                            tile_position=(32 * sub, 0))
                        sl = oe[:, a * D_EMB:(a + 1) * D_EMB]
                        # drains split 24 DVE / 8 ACT: ACT also carries
                        # the mag chain mid-stream
                        if a != 3:
                            nc.vector.tensor_copy(out=sl, in_=po[:])
                        else:
                            nc.scalar.copy(out=sl, in_=po[:])
                    nc.sync.dma_start(
                        out=out[ti0 * 128:(ti0 + gn) * 128, :D_EMB]
                        .rearrange("(a p) e -> p a e", p=128),
                        in_=oe[:].rearrange("p (a e) -> p a e", a=gn))
                    ti0 += gn

                # ---- DFT + mag + projection -- DE-prioritized: the emb
                # drain stream must own DVE/ACT early or the out-DMA
                # starves; this chain only has to deliver S by the time
                # the emb bytes run out (~2/3 into the stream) ----------------
                with tc.high_priority(offset=-1_000_000):
                    S = psp.tile([F, D_STFT], f32, tag="s")
                    first_s = [True]

                    def proj_mm(lhsT, rhs, stop=False):
                        nc.tensor.matmul(out=S[:], lhsT=lhsT, rhs=rhs,
                                         start=first_s[0], stop=stop)
                        first_s[0] = False

                    nyqt = []
                    for ci, (c0, cn) in enumerate(CHUNKS):
                        re = pre.tile([128, cn], f32, tag="re")
                        im = pim.tile([128, cn], f32, tag="im")
                        for c in range(2):
                            cb = c * CWW
                            rhs = OHF[c][:, c0:c0 + cn]
                            nc.tensor.matmul(
                                out=re[:], lhsT=CW[:, cb:cb + 128], rhs=rhs,
                                start=(c == 0), stop=(c == 1))
                            nc.tensor.matmul(
                                out=im[:], lhsT=CW[:, cb + 128:cb + 256],
                                rhs=rhs,
                                start=(c == 0), stop=(c == 1))
                        # t1 = re^2, t2 = im^2 (ACT: DVE cannot read two
                        # PSUM operands). im row 0 carries the nyquist cos
                        # accumulation (sin k=0 is identically 0): read
                        # |ny| off t2 row 0 into NYH (bf16 direct), zero
                        # it, then the full add + sqrt gives |dc| in row 0.
                        t1 = tpool.tile([128, cn], f32, tag="sq1")
                        t2 = tpool.tile([128, cn], f32, tag="sq2")
                        nc.scalar.square(out=t2[:], in_=im[:])
                        nc.scalar.sqrt(out=NYH[:, c0:c0 + cn],
                                       in_=t2[0:1, :])
                        nc.scalar.square(out=t1[:], in_=re[:])
                        nc.vector.memset(t2[0:1, :], 0.0)
                        nc.vector.tensor_tensor(
                            out=t1[:], in0=t1[:], in1=t2[:], op=AO.add)
                        nc.scalar.sqrt(out=MAGH[:, c0:c0 + cn], in_=t1[:])
                        # per-chunk nyquist fold: transpose this chunk's
                        # NYH slice to [nv, F] right after the row sqrt so
                        # the SBUF->SBUF DMA latency overlaps the rest of
                        # the chain; issued on the ACT HWDGE ring (Sync's
                        # queue is crowded with emb out-DMA waits). Chunk
                        # 2 also carries the ones column -> bias row.
                        nv = cn // F + (1 if ci == 2 else 0)
                        nyt = wpool.tile([nv, F], bf16, tag=f"nyqt{ci}")
                        nc.scalar.dma_start(
                            out=nyt[:, :], in_=NYH[:, c0:c0 + nv * F])
                        nyqt.append((nyt, ci, nv))
                        # projection matmuls for this chunk
                        for v in range(c0 // F, (c0 + cn) // F):
                            proj_mm(MAGH[:, v * F:(v + 1) * F],
                                    WP[:, v * D_STFT:(v + 1) * D_STFT])

                    # fold the three transposed nyquist blocks (+bias row)
                    for i, (nyt, ci, nv) in enumerate(nyqt):
                        proj_mm(nyt[:], WN[0:nv,
                                           ci * D_STFT:(ci + 1) * D_STFT],
                                stop=(i == 2))
                    # S -> bf16 into the zero-padded K=128 SH tile
                    nc.vector.tensor_copy(out=SH[:F, :], in_=S[:])

            # ---- stft part of output: B-select @ SH -------------------------
            # two tiles share one PSUM bank (2*154*4B < 2KB): halves the
            # PSUM->SBUF copy op count; drains + DMA issues all on Scalar
            # (the second HWDGE ring), after its mag work is done
            with tc.tile_pool(name="psum_stft", bufs=4, space="PSUM") as pstft:
                ti0 = 0
                for gn in SGROUPS:
                    os_ = ospool.tile([128, gn * D_STFT], f32, tag="os")
                    for half in range(gn // 2):
                        ps = pstft.tile([128, 2 * D_STFT], f32, tag="ps")
                        for sub in range(2):
                            ti = ti0 + half * 2 + sub
                            lhsT = BS[:, ti * 128:(ti + 1) * 128]
                            po = ps[:, sub * D_STFT:(sub + 1) * D_STFT]
                            nc.tensor.matmul(out=po, lhsT=lhsT, rhs=SH[:],
                                             start=True, stop=True)
                        sl = os_[:, half * 2 * D_STFT:(half + 1) * 2 * D_STFT]
                        if half % 2 == 0:
                            nc.scalar.copy(out=sl, in_=ps[:])
                        else:
                            nc.vector.tensor_copy(out=sl, in_=ps[:])
                    nc.sync.dma_start(
                        out=out[ti0 * 128:(ti0 + gn) * 128, D_EMB:DM]
                        .rearrange("(a p) e -> p a e", p=128),
                        in_=os_[:].rearrange("p (a e) -> p a e", a=gn))
                    ti0 += gn

    nc.finalize()
    return nc


def _host_consts():
    import ml_dtypes

    bf16 = ml_dtypes.bfloat16
    n = np.arange(NFFT)
    window = 0.5 - 0.5 * np.cos(2.0 * np.pi * n / NFFT)
    k = np.arange(NFREQ)
    ang = 2.0 * np.pi * np.outer(n, k) / NFFT  # (256, 129)
    wcos = (window[:, None] * np.cos(ang)).astype(np.float32)
    wsin = (window[:, None] * np.sin(ang)).astype(np.float32)
    CWW = 2 * 128
    cwf = np.zeros((128, 2 * CWW), np.float32)
    for c in range(2):
        rows = slice(c * 128, (c + 1) * 128)
        # block layout per c: [cos k0..k127 | nyq cos, sin k1..k127]
        blk = np.zeros((128, CWW), np.float32)
        blk[:, :128] = wcos[rows, :128]
        blk[:, 128] = wcos[rows][:, 128]  # nyquist cos -> im column 0
        blk[:, 129:256] = wsin[rows, 1:128]
        cwf[:, c * CWW:(c + 1) * CWW] = blk
    cw = cwf.astype(bf16)

    vfr = np.broadcast_to(np.arange(V, dtype=np.int8), (128, V)).copy()
    vemb = (np.arange(128, dtype=np.int8) % 32).reshape(128, 1).copy()
    return cw, vfr, vemb


def _bsel_for_half(h):
    import ml_dtypes

    t = np.arange(LH)
    fglob = (129 * (t + LH * h)) >> 13
    floc = fglob - 64 * h
    bs = np.zeros((128, LH), np.float32)
    bs[floc, t] = 1.0
    return bs.astype(ml_dtypes.bfloat16)


def kernel(input_ids, emb_table, proj_w, proj_b):
    global _PROG, LAST_RESULT
    import ml_dtypes

    from concourse.bass_utils import run_bass_kernel_spmd

    bf16 = ml_dtypes.bfloat16
    ids = np.asarray(input_ids).astype(np.int32)
    emb = np.asarray(emb_table).astype(np.float32)
    pw = np.asarray(proj_w).astype(np.float32)
    pb = np.asarray(proj_b).astype(np.float32)

    cw, vfr, vemb = _host_consts()

    # proj_w rows are indexed by i = k*V + v (freq-major)
    wproj = np.zeros((128, V * D_STFT), np.float32)
    for v in range(V):
        wproj[:, v * D_STFT:(v + 1) * D_STFT] = pw[np.arange(128) * V + v]
    wproj = wproj.astype(bf16)
    # nyquist proj weights in 3 per-chunk blocks, each starting at
    # partition 0: [v0..6 | v7..13 | v14, v15, bias]
    nyw = pw[128 * V + np.arange(V)]  # (16, 154)
    wnyq = np.zeros((7, 3 * D_STFT), np.float32)
    wnyq[0:7, :D_STFT] = nyw[0:7]
    wnyq[0:7, D_STFT:2 * D_STFT] = nyw[7:14]
    wnyq[0:2, 2 * D_STFT:] = nyw[14:16]
    wnyq[2, 2 * D_STFT:] = pb.reshape(1, D_STFT)
    wnyq = wnyq.astype(bf16)

    embrep = np.zeros((128, D_EMB), np.float32)
    for a in range(4):
        embrep[32 * a:32 * a + V] = emb
    embrep = embrep.astype(bf16)

    # apack: [embrep | cw | wnyq blocks] (bf16)
    apack = np.zeros((128, D_EMB + cw.shape[1] + 3 * D_STFT), bf16)
    apack[:, :D_EMB] = embrep
    apack[:, D_EMB:D_EMB + cw.shape[1]] = cw
    apack[:7, D_EMB + cw.shape[1]:] = wnyq

    bsel = [_bsel_for_half(h) for h in range(2)]

    in_maps = []
    for core in range(NCORES):
        b, h = divmod(core, 2)
        padded = np.pad(ids[b], 128, mode="reflect")
        seg = padded[LH * h:LH * h + 64 * (F - 1) + NFFT]  # (4352,)
        hf = np.zeros((128, 2 * F), np.int8)
        for c in range(2):
            idx = (64 * np.arange(F)[None, :] + 128 * c
                   + np.arange(128)[:, None])
            hf[:, c * F:(c + 1) * F] = seg[idx]
        ids_out = ids[b, LH * h:LH * (h + 1)]
        he = np.zeros((128, LH // 4), np.int8)
        tiles = ids_out.reshape(NT, 128)  # tile ti = 4q+a
        for a in range(4):
            rows = tiles[a::4]  # (8, 128), q-major
            he[32 * a:32 * a + V, :] = np.broadcast_to(
                rows.reshape(1, LH // 4), (V, LH // 4))
        # ipack: [h_emb | vemb | h_frames | vfr] (int8)
        ipack = np.concatenate([he, vemb, hf, vfr], axis=1)
        in_maps.append({
            "ipack": ipack, "apack": apack, "bsel": bsel[h], "wproj": wproj,
        })

    if _PROG is None:
        _PROG = _build_program()

    res = run_bass_kernel_spmd(_PROG, in_maps, core_ids=list(range(NCORES)))
    LAST_RESULT = res

    full = np.zeros((B, L, DM), np.float32)
    for core in range(NCORES):
        b, h = divmod(core, 2)
        full[b, LH * h:LH * (h + 1), :] = res.results[core]["out"]
    return full

